# revision 46
# baseline (speedup 1.0000x reference)
"""Trainium2 Bass kernel for the masked ("permuted") GRU cell.

Math (reference):
    Wg_masked = Wg * tril(ones(H,H))        for all six [H,H] weights
    gi = x @ [Wir_m | Wiz_m | Win_m]        [B, 3H]
    gh = h @ [Whr_m | Whz_m | Whn_m]        [B, 3H]
    r  = sigmoid(i_r + h_r + b_hr)
    z  = sigmoid(i_z + h_z + b_hz)
    n  = tanh(i_n + r * (h_n + b_hn))
    hy = h * z + (1 - z) * n

Strategy (8 NeuronCores, SPMD, data-parallel over batch):
  * Each core takes a 512-row batch shard; weights replicated.
  * All matmul operands are fp16 (1 PE cycle/row like fp32r, half the HBM
    bytes, and ~8x less quantization error than bf16); accumulation in
    fp32 PSUM, gate math + output in fp32.
  * The host does all layout prep so the device does nothing but matmuls
    and gate math:
      - the six weights are masked by tril, cast to fp16, and packed into
        one [128, 104448] buffer holding exactly the 136 surviving 128x128
        tiles per gate, grouped by output block jb (descending, the
        execution order).  One DMA instruction per jb streams all six
        gates' strips with 128 descriptors of up to 24KB contiguous bytes
        (vs ~100k 512B descriptors when gathering from the [H,H] layout).
      - x and hidden arrive pre-transposed ([H, 512] per core) in fp16, so
        no on-chip TensorE transposes are needed; the fp16 hidden slices
        also feed the final blend (h is only ever multiplied by z, so fp16
        h costs ~5e-4 relative there).
      - biases arrive pre-transposed as [128, 48] per-partition columns.
  * Per jb the four PSUM groups (r, z, i_n, h_n) accumulate over
    k = jb..15; groups are double-buffered (8 banks) so jb+1's matmuls
    overlap jb's gate math on the Vector/Scalar engines.
  * Output is stored transposed ([H, 512] fp32) and untransposed on host.
"""

import numpy as np
from contextlib import ExitStack

import concourse.bass as bass
from concourse import bacc
import concourse.mybir as mybir
import concourse.tile as tile
from concourse.bass_utils import run_bass_kernel_spmd

B = 4096
H = 2048
NCORES = 8
BS = B // NCORES          # batch rows per core = 512
P = 128                   # partition dim / tile edge
KT = H // P               # 16 k (and j) tiles
FD = BS                   # moving free dim = per-core batch = 512
F32 = mybir.dt.float32
F16 = mybir.dt.float16
AF = mybir.ActivationFunctionType

# matmul chain order per jb: i_n and h_n groups first so the n-path gate
# math can start while the r/z chains are still streaming.
G_ORDER = ["W_in", "W_hn", "W_ir", "W_hr", "W_iz", "W_hz"]
B_NAMES = ["b_hr", "b_hz", "b_hn"]

# jb-major pack offsets (execution order: jb = 15 .. 0)
JB_ORDER = list(reversed(range(KT)))
_PACK_OFF = {}
_off = 0
for _jb in JB_ORDER:
    _PACK_OFF[_jb] = _off
    _off += 6 * (KT - _jb) * P
WCOLS = _off              # 6 * 136 * 128 = 104448


def _emit(ctx: ExitStack, tc: "tile.TileContext"):
    nc = tc.nc

    xT = nc.dram_tensor("xT", [H, FD], F16, kind="ExternalInput").ap()
    hT = nc.dram_tensor("hT", [H, FD], F16, kind="ExternalInput").ap()
    wpk = nc.dram_tensor("wpack", [P, WCOLS], F16, kind="ExternalInput").ap()
    bias = nc.dram_tensor("biasT", [P, 3 * KT], F32, kind="ExternalInput").ap()
    hyT = nc.dram_tensor("hyT", [H, FD], F32, kind="ExternalOutput").ap()

    iopool = ctx.enter_context(tc.tile_pool(name="io", bufs=1))
    wpool = ctx.enter_context(tc.tile_pool(name="w", bufs=4))
    epool = ctx.enter_context(tc.tile_pool(name="ew", bufs=2))
    pspool = ctx.enter_context(tc.tile_pool(name="ps", bufs=2, space="PSUM"))

    bias_sb = iopool.tile([P, 3 * KT], F32, tag="bias")

    # PE warm-up: small throwaway matmuls while the first weight/input
    # DMAs are in flight, so the PE p-state is ramped when the real
    # stream starts.  Small free dim (128) so they never delay it.
    dummy = iopool.tile([P, P], F16, tag="dummy")
    nc.vector.memset(dummy[:], 0)
    warm = pspool.tile([P, FD], F32, tag="ps_i")
    for _ in range(26):
        nc.tensor.matmul(warm[:, :P], dummy[:], dummy[:], start=True, stop=True)

    # per-k input tiles (separate tiles so the first matmuls only wait on
    # their own k slice), DMA'd interleaved with the weight strips in
    # consumption order.  The bias is only needed by the first sigmoid, so
    # it is issued after the first step's weights/inputs.
    xk = [iopool.tile([P, FD], F16, tag=f"x_{k}", name=f"x_{k}")
          for k in range(KT)]
    hk = [iopool.tile([P, FD], F16, tag=f"h_{k}", name=f"h_{k}")
          for k in range(KT)]

    # Descriptor generation costs ~0.6us of sequencer time per dma_start,
    # so split the issue load: weight strips on the sync engine (1/step),
    # x/h slices on the scalar engine, 3 steps ahead of their use.
    def load_xh(k):
        nc.scalar.dma_start(xk[k][:], xT[k * P:(k + 1) * P, :])
        nc.scalar.dma_start(hk[k][:], hT[k * P:(k + 1) * P, :])

    wts = {}
    for jb in JB_ORDER:
        nk = KT - jb
        wt = wpool.tile([P, 6 * KT * P], F16, tag="w")
        nc.sync.dma_start(wt[:, :6 * nk * P],
                          wpk[:, _PACK_OFF[jb]:_PACK_OFF[jb] + 6 * nk * P])
        wts[jb] = wt

    for k in JB_ORDER[:3]:
        load_xh(k)
    nc.scalar.dma_start(bias_sb[:], bias)

    for s, jb in enumerate(JB_ORDER):
        nk = KT - jb
        wt = wts[jb]
        if s + 3 < KT:
            load_xh(JB_ORDER[s + 3])

        def lhsT(gi_, k):
            c0 = (gi_ * nk + (k - jb)) * P
            return wt[:, c0:c0 + P]

        psi = pspool.tile([P, FD], F32, tag="ps_i")
        psh = pspool.tile([P, FD], F32, tag="ps_h")
        psr = pspool.tile([P, FD], F32, tag="ps_r")
        psz = pspool.tile([P, FD], F32, tag="ps_z")

        def chain(ps, gi_, src, start, stop):
            for i, k in enumerate(range(jb, KT)):
                nc.tensor.matmul(ps[:], lhsT(gi_, k), src[k][:],
                                 start=start and i == 0,
                                 stop=stop and i == nk - 1)

        chain(psi, 0, xk, True, True)    # i_n
        chain(psh, 1, hk, True, True)    # h_n
        chain(psr, 2, xk, True, False)   # i_r
        chain(psr, 3, hk, False, True)   # + h_r
        chain(psz, 4, xk, True, False)   # i_z
        chain(psz, 5, hk, False, True)   # + h_z

        # gates; all tiles are [j=128, b=512] fp32
        t_sb = epool.tile([P, FD], F32, tag="t")
        nc.vector.tensor_scalar_add(t_sb[:], psh[:],
                                    bias_sb[:, 2 * KT + jb:2 * KT + jb + 1])
        r_sb = epool.tile([P, FD], F32, tag="r")
        nc.scalar.activation(r_sb[:], psr[:], AF.Sigmoid,
                             bias=bias_sb[:, jb:jb + 1])
        nc.vector.tensor_mul(t_sb[:], t_sb[:], r_sb[:])
        nc.vector.tensor_add(t_sb[:], t_sb[:], psi[:])
        n_sb = epool.tile([P, FD], F32, tag="n")
        nc.scalar.activation(n_sb[:], t_sb[:], AF.Tanh)
        o_sb = epool.tile([P, FD], F32, tag="o")
        nc.vector.tensor_sub(o_sb[:], hk[jb][:], n_sb[:])

        # z path + blend + store.  On the final jb this is the post-stream
        # serial tail, so run it in two batch halves to pipeline the
        # sigmoid (Scalar) against the blend (Vector) against the DMA.
        z_sb = epool.tile([P, FD], F32, tag="z")
        halves = [(0, FD // 2), (FD // 2, FD)] if jb == JB_ORDER[-1] \
            else [(0, FD)]
        for (c0, c1) in halves:
            nc.scalar.activation(z_sb[:, c0:c1], psz[:, c0:c1], AF.Sigmoid,
                                 bias=bias_sb[:, KT + jb:KT + jb + 1])
            # hy = n + z * (h - n)
            nc.vector.tensor_mul(o_sb[:, c0:c1], o_sb[:, c0:c1],
                                 z_sb[:, c0:c1])
            nc.vector.tensor_add(o_sb[:, c0:c1], o_sb[:, c0:c1],
                                 n_sb[:, c0:c1])
            nc.sync.dma_start(hyT[jb * P:(jb + 1) * P, c0:c1],
                              o_sb[:, c0:c1])


_CACHE = {}


def _program():
    if "nc" not in _CACHE:
        nc = bacc.Bacc()
        with tile.TileContext(nc) as tc:
            with ExitStack() as ctx:
                _emit(ctx, tc)
        nc.compile()
        _CACHE["nc"] = nc
    return _CACHE["nc"]


def _in_maps(inputs):
    x = np.ascontiguousarray(inputs["x"], dtype=np.float32)
    h = np.ascontiguousarray(inputs["hidden"], dtype=np.float32)
    lower = np.asarray(inputs["lower"], dtype=np.float32)

    # jb-major fp16 weight pack (see module docstring)
    pack = np.empty((P, WCOLS), dtype=np.float16)
    masked = {g: (np.asarray(inputs[g], np.float32) * lower).astype(np.float16)
              for g in G_ORDER}
    for jb in JB_ORDER:
        nk = KT - jb
        off = _PACK_OFF[jb]
        for gi_, g in enumerate(G_ORDER):
            # [nk*128, 128] block of masked weight, tiled to [128, nk*128]
            blk = masked[g][jb * P:, jb * P:(jb + 1) * P]
            blk = blk.reshape(nk, P, P).transpose(1, 0, 2).reshape(P, nk * P)
            pack[:, off + gi_ * nk * P: off + (gi_ + 1) * nk * P] = blk

    biasT = np.concatenate(
        [np.asarray(inputs[n], np.float32).reshape(KT, P).T for n in B_NAMES],
        axis=1)
    biasT = np.ascontiguousarray(biasT)

    x16 = x.astype(np.float16)
    h16 = h.astype(np.float16)
    maps = []
    for c in range(NCORES):
        sl = slice(c * BS, (c + 1) * BS)
        maps.append({
            "xT": np.ascontiguousarray(x16[sl].T),
            "hT": np.ascontiguousarray(h16[sl].T),
            "wpack": pack,
            "biasT": biasT,
        })
    return maps


def run(inputs, **kw):
    nc = _program()
    res = run_bass_kernel_spmd(nc, _in_maps(inputs), list(range(NCORES)), **kw)
    out = np.empty((B, H), dtype=np.float32)
    for c in range(NCORES):
        out[c * BS:(c + 1) * BS, :] = res.results[c]["hyT"].T
    return out, res


def kernel(**inputs) -> np.ndarray:
    out, _ = run(inputs)
    return out


# revision 47
# speedup vs baseline: 1.0190x; 1.0190x over previous
"""Trainium2 Bass kernel for the masked ("permuted") GRU cell.

Math (reference):
    Wg_masked = Wg * tril(ones(H,H))        for all six [H,H] weights
    gi = x @ [Wir_m | Wiz_m | Win_m]        [B, 3H]
    gh = h @ [Whr_m | Whz_m | Whn_m]        [B, 3H]
    r  = sigmoid(i_r + h_r + b_hr)
    z  = sigmoid(i_z + h_z + b_hz)
    n  = tanh(i_n + r * (h_n + b_hn))
    hy = h * z + (1 - z) * n

Strategy (8 NeuronCores, SPMD, data-parallel over batch):
  * Each core takes a 512-row batch shard; weights replicated.
  * All matmul operands are fp16 (1 PE cycle/row like fp32r, half the HBM
    bytes, and ~8x less quantization error than bf16); accumulation in
    fp32 PSUM, gate math + output in fp32.
  * The host does all layout prep so the device does nothing but matmuls
    and gate math:
      - the six weights are masked by tril, cast to fp16, and packed into
        one [128, 104448] buffer holding exactly the 136 surviving 128x128
        tiles per gate, grouped by output block jb (descending, the
        execution order).  One DMA instruction per jb streams all six
        gates' strips with 128 descriptors of up to 24KB contiguous bytes
        (vs ~100k 512B descriptors when gathering from the [H,H] layout).
      - x and hidden arrive pre-transposed ([H, 512] per core) in fp16, so
        no on-chip TensorE transposes are needed; the fp16 hidden slices
        also feed the final blend (h is only ever multiplied by z, so fp16
        h costs ~5e-4 relative there).
      - biases arrive pre-transposed as [128, 48] per-partition columns.
  * Per jb the four PSUM groups (r, z, i_n, h_n) accumulate over
    k = jb..15; groups are double-buffered (8 banks) so jb+1's matmuls
    overlap jb's gate math on the Vector/Scalar engines.
  * Output is stored transposed ([H, 512] fp32) and untransposed on host.
"""

import numpy as np
from contextlib import ExitStack

import concourse.bass as bass
from concourse import bacc
import concourse.mybir as mybir
import concourse.tile as tile
from concourse.bass_utils import run_bass_kernel_spmd

B = 4096
H = 2048
NCORES = 8
BS = B // NCORES          # batch rows per core = 512
P = 128                   # partition dim / tile edge
KT = H // P               # 16 k (and j) tiles
FD = BS                   # moving free dim = per-core batch = 512
F32 = mybir.dt.float32
F16 = mybir.dt.float16
AF = mybir.ActivationFunctionType

# matmul chain order per jb: i_n and h_n groups first so the n-path gate
# math can start while the r/z chains are still streaming.
G_ORDER = ["W_in", "W_hn", "W_ir", "W_hr", "W_iz", "W_hz"]
B_NAMES = ["b_hr", "b_hz", "b_hn"]

# jb-major pack offsets (execution order: jb = 15 .. 0)
JB_ORDER = list(reversed(range(KT)))
_PACK_OFF = {}
_off = 0
for _jb in JB_ORDER:
    _PACK_OFF[_jb] = _off
    _off += 6 * (KT - _jb) * P
WCOLS = _off              # 6 * 136 * 128 = 104448


def _emit(ctx: ExitStack, tc: "tile.TileContext"):
    nc = tc.nc

    xT = nc.dram_tensor("xT", [H, FD], F16, kind="ExternalInput").ap()
    hT = nc.dram_tensor("hT", [H, FD], F16, kind="ExternalInput").ap()
    wpk = nc.dram_tensor("wpack", [P, WCOLS], F16, kind="ExternalInput").ap()
    bias = nc.dram_tensor("biasT", [P, 3 * KT], F32, kind="ExternalInput").ap()
    hyT = nc.dram_tensor("hyT", [H, FD], F32, kind="ExternalOutput").ap()

    iopool = ctx.enter_context(tc.tile_pool(name="io", bufs=1))
    wpool = ctx.enter_context(tc.tile_pool(name="w", bufs=4))
    epool = ctx.enter_context(tc.tile_pool(name="ew", bufs=2))
    pspool = ctx.enter_context(tc.tile_pool(name="ps", bufs=2, space="PSUM"))

    bias_sb = iopool.tile([P, 3 * KT], F32, tag="bias")

    # PE warm-up: small throwaway matmuls while the first weight/input
    # DMAs are in flight, so the PE p-state is ramped when the real
    # stream starts.  Small free dim (128) so they never delay it.
    dummy = iopool.tile([P, P], F16, tag="dummy")
    nc.vector.memset(dummy[:], 0)
    warm = pspool.tile([P, FD], F32, tag="ps_i")
    for _ in range(26):
        nc.tensor.matmul(warm[:, :P], dummy[:], dummy[:], start=True, stop=True)

    # per-k input tiles (separate tiles so the first matmuls only wait on
    # their own k slice), DMA'd interleaved with the weight strips in
    # consumption order.  The bias is only needed by the first sigmoid, so
    # it is issued after the first step's weights/inputs.
    xk = [iopool.tile([P, FD], F16, tag=f"x_{k}", name=f"x_{k}")
          for k in range(KT)]
    hk = [iopool.tile([P, FD], F16, tag=f"h_{k}", name=f"h_{k}")
          for k in range(KT)]

    wts = {}
    for jb in JB_ORDER:
        nk = KT - jb
        wt = wpool.tile([P, 6 * KT * P], F16, tag="w")
        nc.sync.dma_start(wt[:, :6 * nk * P],
                          wpk[:, _PACK_OFF[jb]:_PACK_OFF[jb] + 6 * nk * P])
        wts[jb] = wt
        k = jb
        nc.sync.dma_start(xk[k][:], xT[k * P:(k + 1) * P, :])
        nc.sync.dma_start(hk[k][:], hT[k * P:(k + 1) * P, :])
        if jb == JB_ORDER[0]:
            nc.sync.dma_start(bias_sb[:], bias)

    for s, jb in enumerate(JB_ORDER):
        nk = KT - jb
        wt = wts[jb]

        def lhsT(gi_, k):
            c0 = (gi_ * nk + (k - jb)) * P
            return wt[:, c0:c0 + P]

        psi = pspool.tile([P, FD], F32, tag="ps_i")
        psh = pspool.tile([P, FD], F32, tag="ps_h")
        psr = pspool.tile([P, FD], F32, tag="ps_r")
        psz = pspool.tile([P, FD], F32, tag="ps_z")

        def chain(ps, gi_, src, start, stop):
            for i, k in enumerate(range(jb, KT)):
                nc.tensor.matmul(ps[:], lhsT(gi_, k), src[k][:],
                                 start=start and i == 0,
                                 stop=stop and i == nk - 1)

        chain(psi, 0, xk, True, True)    # i_n
        chain(psh, 1, hk, True, True)    # h_n
        chain(psr, 2, xk, True, False)   # i_r
        chain(psr, 3, hk, False, True)   # + h_r
        chain(psz, 4, xk, True, False)   # i_z
        chain(psz, 5, hk, False, True)   # + h_z

        # gates; all tiles are [j=128, b=512] fp32
        t_sb = epool.tile([P, FD], F32, tag="t")
        nc.vector.tensor_scalar_add(t_sb[:], psh[:],
                                    bias_sb[:, 2 * KT + jb:2 * KT + jb + 1])
        r_sb = epool.tile([P, FD], F32, tag="r")
        nc.scalar.activation(r_sb[:], psr[:], AF.Sigmoid,
                             bias=bias_sb[:, jb:jb + 1])
        nc.vector.tensor_mul(t_sb[:], t_sb[:], r_sb[:])
        nc.vector.tensor_add(t_sb[:], t_sb[:], psi[:])
        n_sb = epool.tile([P, FD], F32, tag="n")
        nc.scalar.activation(n_sb[:], t_sb[:], AF.Tanh)
        o_sb = epool.tile([P, FD], F32, tag="o")
        nc.vector.tensor_sub(o_sb[:], hk[jb][:], n_sb[:])

        # z path + blend + store.  On the final jb this is the post-stream
        # serial tail, so run it in two batch halves to pipeline the
        # sigmoid (Scalar) against the blend (Vector) against the DMA.
        z_sb = epool.tile([P, FD], F32, tag="z")
        halves = [(0, FD // 2), (FD // 2, FD)] if jb == JB_ORDER[-1] \
            else [(0, FD)]
        for (c0, c1) in halves:
            nc.scalar.activation(z_sb[:, c0:c1], psz[:, c0:c1], AF.Sigmoid,
                                 bias=bias_sb[:, KT + jb:KT + jb + 1])
            # hy = n + z * (h - n)
            nc.vector.tensor_mul(o_sb[:, c0:c1], o_sb[:, c0:c1],
                                 z_sb[:, c0:c1])
            nc.vector.tensor_add(o_sb[:, c0:c1], o_sb[:, c0:c1],
                                 n_sb[:, c0:c1])
            nc.sync.dma_start(hyT[jb * P:(jb + 1) * P, c0:c1],
                              o_sb[:, c0:c1])


_CACHE = {}


def _program():
    if "nc" not in _CACHE:
        nc = bacc.Bacc()
        with tile.TileContext(nc) as tc:
            with ExitStack() as ctx:
                _emit(ctx, tc)
        nc.compile()
        _CACHE["nc"] = nc
    return _CACHE["nc"]


def _in_maps(inputs):
    x = np.ascontiguousarray(inputs["x"], dtype=np.float32)
    h = np.ascontiguousarray(inputs["hidden"], dtype=np.float32)
    lower = np.asarray(inputs["lower"], dtype=np.float32)

    # jb-major fp16 weight pack (see module docstring)
    pack = np.empty((P, WCOLS), dtype=np.float16)
    masked = {g: (np.asarray(inputs[g], np.float32) * lower).astype(np.float16)
              for g in G_ORDER}
    for jb in JB_ORDER:
        nk = KT - jb
        off = _PACK_OFF[jb]
        for gi_, g in enumerate(G_ORDER):
            # [nk*128, 128] block of masked weight, tiled to [128, nk*128]
            blk = masked[g][jb * P:, jb * P:(jb + 1) * P]
            blk = blk.reshape(nk, P, P).transpose(1, 0, 2).reshape(P, nk * P)
            pack[:, off + gi_ * nk * P: off + (gi_ + 1) * nk * P] = blk

    biasT = np.concatenate(
        [np.asarray(inputs[n], np.float32).reshape(KT, P).T for n in B_NAMES],
        axis=1)
    biasT = np.ascontiguousarray(biasT)

    x16 = x.astype(np.float16)
    h16 = h.astype(np.float16)
    maps = []
    for c in range(NCORES):
        sl = slice(c * BS, (c + 1) * BS)
        maps.append({
            "xT": np.ascontiguousarray(x16[sl].T),
            "hT": np.ascontiguousarray(h16[sl].T),
            "wpack": pack,
            "biasT": biasT,
        })
    return maps


def run(inputs, **kw):
    nc = _program()
    res = run_bass_kernel_spmd(nc, _in_maps(inputs), list(range(NCORES)), **kw)
    out = np.empty((B, H), dtype=np.float32)
    for c in range(NCORES):
        out[c * BS:(c + 1) * BS, :] = res.results[c]["hyT"].T
    return out, res


def kernel(**inputs) -> np.ndarray:
    out, _ = run(inputs)
    return out


# revision 52
# speedup vs baseline: 1.0263x; 1.0071x over previous
"""Trainium2 Bass kernel for the masked ("permuted") GRU cell.

Math (reference):
    Wg_masked = Wg * tril(ones(H,H))        for all six [H,H] weights
    gi = x @ [Wir_m | Wiz_m | Win_m]        [B, 3H]
    gh = h @ [Whr_m | Whz_m | Whn_m]        [B, 3H]
    r  = sigmoid(i_r + h_r + b_hr)
    z  = sigmoid(i_z + h_z + b_hz)
    n  = tanh(i_n + r * (h_n + b_hn))
    hy = h * z + (1 - z) * n

Strategy (8 NeuronCores, SPMD, data-parallel over batch):
  * Each core takes a 512-row batch shard; weights replicated.
  * All matmul operands are fp16 (1 PE cycle/row like fp32r, half the HBM
    bytes, and ~8x less quantization error than bf16); accumulation in
    fp32 PSUM, gate math + output in fp32.
  * The host does all layout prep so the device does nothing but matmuls
    and gate math:
      - the six weights are masked by tril, cast to fp16, and packed into
        one [128, 104448] buffer holding exactly the 136 surviving 128x128
        tiles per gate, grouped by output block jb (descending, the
        execution order).  One DMA instruction per jb streams all six
        gates' strips with 128 descriptors of up to 24KB contiguous bytes
        (vs ~100k 512B descriptors when gathering from the [H,H] layout).
      - x and hidden arrive pre-transposed ([H, 512] per core) in fp16, so
        no on-chip TensorE transposes are needed; the fp16 hidden slices
        also feed the final blend (h is only ever multiplied by z, so fp16
        h costs ~5e-4 relative there).
      - biases arrive pre-transposed as [128, 48] per-partition columns.
  * Per jb the four PSUM groups (r, z, i_n, h_n) accumulate over
    k = jb..15; groups are double-buffered (8 banks) so jb+1's matmuls
    overlap jb's gate math on the Vector/Scalar engines.
  * Output is stored transposed ([H, 512] fp32) and untransposed on host.
"""

import numpy as np
from contextlib import ExitStack

import concourse.bass as bass
from concourse import bacc
import concourse.mybir as mybir
import concourse.tile as tile
from concourse.bass_utils import run_bass_kernel_spmd

B = 4096
H = 2048
NCORES = 8
BS = B // NCORES          # batch rows per core = 512
P = 128                   # partition dim / tile edge
KT = H // P               # 16 k (and j) tiles
FD = BS                   # moving free dim = per-core batch = 512
F32 = mybir.dt.float32
F16 = mybir.dt.float16
AF = mybir.ActivationFunctionType

# matmul chain order per jb: i_n and h_n groups first so the n-path gate
# math can start while the r/z chains are still streaming.
G_ORDER = ["W_in", "W_hn", "W_ir", "W_hr", "W_iz", "W_hz"]
B_NAMES = ["b_hr", "b_hz", "b_hn"]

# jb-major pack offsets (execution order: jb = 15 .. 0)
JB_ORDER = list(reversed(range(KT)))
_PACK_OFF = {}
_off = 0
for _jb in JB_ORDER:
    _PACK_OFF[_jb] = _off
    _off += 6 * (KT - _jb) * P
WCOLS = _off              # 6 * 136 * 128 = 104448


def _emit(ctx: ExitStack, tc: "tile.TileContext"):
    nc = tc.nc

    # x and h pre-transposed and packed side by side: row k*128+p holds
    # [ x.T[k*128+p, :512] | h.T[k*128+p, :512] ] so one DMA per k slice
    # moves both with 2KB descriptor lines.
    xhT = nc.dram_tensor("xhT", [H, 2 * FD], F16, kind="ExternalInput").ap()
    wpk = nc.dram_tensor("wpack", [P, WCOLS], F16, kind="ExternalInput").ap()
    bias = nc.dram_tensor("biasT", [P, 3 * KT], F32, kind="ExternalInput").ap()
    hyT = nc.dram_tensor("hyT", [H, FD], F32, kind="ExternalOutput").ap()

    iopool = ctx.enter_context(tc.tile_pool(name="io", bufs=1))
    wpool = ctx.enter_context(tc.tile_pool(name="w", bufs=4))
    epool = ctx.enter_context(tc.tile_pool(name="ew", bufs=2))
    pspool = ctx.enter_context(tc.tile_pool(name="ps", bufs=2, space="PSUM"))

    bias_sb = iopool.tile([P, 3 * KT], F32, tag="bias")

    # PE warm-up: small throwaway matmuls while the first weight/input
    # DMAs are in flight, so the PE p-state is ramped when the real
    # stream starts.  Small free dim (128) so they never delay it.
    dummy = iopool.tile([P, P], F16, tag="dummy")
    nc.vector.memset(dummy[:], 0)
    warm = pspool.tile([P, FD], F32, tag="ps_i")
    for _ in range(26):
        nc.tensor.matmul(warm[:, :P], dummy[:], dummy[:], start=True, stop=True)

    # per-k input tiles (separate tiles so the first matmuls only wait on
    # their own k slice), DMA'd interleaved with the weight strips in
    # consumption order.  The bias is only needed by the first sigmoid, so
    # it is issued after the first step's weights/inputs.
    xhk = [iopool.tile([P, 2 * FD], F16, tag=f"xh_{k}", name=f"xh_{k}")
           for k in range(KT)]
    xk = [t[:, :FD] for t in xhk]
    hk = [t[:, FD:] for t in xhk]

    wts = {}
    for jb in JB_ORDER:
        nk = KT - jb
        wt = wpool.tile([P, 6 * KT * P], F16, tag="w")
        nc.sync.dma_start(wt[:, :6 * nk * P],
                          wpk[:, _PACK_OFF[jb]:_PACK_OFF[jb] + 6 * nk * P])
        wts[jb] = wt
        nc.sync.dma_start(xhk[jb][:], xhT[jb * P:(jb + 1) * P, :])
        if jb == JB_ORDER[1]:
            nc.sync.dma_start(bias_sb[:], bias)

    for s, jb in enumerate(JB_ORDER):
        nk = KT - jb
        wt = wts[jb]

        def lhsT(gi_, k):
            c0 = (gi_ * nk + (k - jb)) * P
            return wt[:, c0:c0 + P]

        psi = pspool.tile([P, FD], F32, tag="ps_i")
        psh = pspool.tile([P, FD], F32, tag="ps_h")
        psr = pspool.tile([P, FD], F32, tag="ps_r")
        psz = pspool.tile([P, FD], F32, tag="ps_z")

        def chain(ps, gi_, src, start, stop):
            for i, k in enumerate(range(jb, KT)):
                nc.tensor.matmul(ps[:], lhsT(gi_, k), src[k],
                                 start=start and i == 0,
                                 stop=stop and i == nk - 1)

        chain(psi, 0, xk, True, True)    # i_n
        chain(psh, 1, hk, True, True)    # h_n
        chain(psr, 2, xk, True, False)   # i_r
        chain(psr, 3, hk, False, True)   # + h_r
        chain(psz, 4, xk, True, False)   # i_z
        chain(psz, 5, hk, False, True)   # + h_z

        # gates; all tiles are [j=128, b=512] fp32
        t_sb = epool.tile([P, FD], F32, tag="t")
        nc.vector.tensor_scalar_add(t_sb[:], psh[:],
                                    bias_sb[:, 2 * KT + jb:2 * KT + jb + 1])
        r_sb = epool.tile([P, FD], F32, tag="r")
        nc.scalar.activation(r_sb[:], psr[:], AF.Sigmoid,
                             bias=bias_sb[:, jb:jb + 1])
        nc.vector.tensor_mul(t_sb[:], t_sb[:], r_sb[:])
        nc.vector.tensor_add(t_sb[:], t_sb[:], psi[:])
        n_sb = epool.tile([P, FD], F32, tag="n")
        nc.scalar.activation(n_sb[:], t_sb[:], AF.Tanh)
        o_sb = epool.tile([P, FD], F32, tag="o")
        nc.vector.tensor_sub(o_sb[:], hk[jb], n_sb[:])

        # z path + blend + store.  On the final jb this is the post-stream
        # serial tail, so run it in two batch halves to pipeline the
        # sigmoid (Scalar) against the blend (Vector) against the DMA.
        z_sb = epool.tile([P, FD], F32, tag="z")
        halves = [(0, FD // 2), (FD // 2, FD)] if jb == JB_ORDER[-1] \
            else [(0, FD)]
        for (c0, c1) in halves:
            nc.scalar.activation(z_sb[:, c0:c1], psz[:, c0:c1], AF.Sigmoid,
                                 bias=bias_sb[:, KT + jb:KT + jb + 1])
            # hy = n + z * (h - n)
            nc.vector.tensor_mul(o_sb[:, c0:c1], o_sb[:, c0:c1],
                                 z_sb[:, c0:c1])
            nc.vector.tensor_add(o_sb[:, c0:c1], o_sb[:, c0:c1],
                                 n_sb[:, c0:c1])
            nc.sync.dma_start(hyT[jb * P:(jb + 1) * P, c0:c1],
                              o_sb[:, c0:c1])


_CACHE = {}


def _program():
    if "nc" not in _CACHE:
        nc = bacc.Bacc()
        with tile.TileContext(nc) as tc:
            with ExitStack() as ctx:
                _emit(ctx, tc)
        nc.compile()
        _CACHE["nc"] = nc
    return _CACHE["nc"]


def _in_maps(inputs):
    x = np.ascontiguousarray(inputs["x"], dtype=np.float32)
    h = np.ascontiguousarray(inputs["hidden"], dtype=np.float32)
    lower = np.asarray(inputs["lower"], dtype=np.float32)

    # jb-major fp16 weight pack (see module docstring)
    pack = np.empty((P, WCOLS), dtype=np.float16)
    masked = {g: (np.asarray(inputs[g], np.float32) * lower).astype(np.float16)
              for g in G_ORDER}
    for jb in JB_ORDER:
        nk = KT - jb
        off = _PACK_OFF[jb]
        for gi_, g in enumerate(G_ORDER):
            # [nk*128, 128] block of masked weight, tiled to [128, nk*128]
            blk = masked[g][jb * P:, jb * P:(jb + 1) * P]
            blk = blk.reshape(nk, P, P).transpose(1, 0, 2).reshape(P, nk * P)
            pack[:, off + gi_ * nk * P: off + (gi_ + 1) * nk * P] = blk

    biasT = np.concatenate(
        [np.asarray(inputs[n], np.float32).reshape(KT, P).T for n in B_NAMES],
        axis=1)
    biasT = np.ascontiguousarray(biasT)

    x16 = x.astype(np.float16)
    h16 = h.astype(np.float16)
    maps = []
    for c in range(NCORES):
        sl = slice(c * BS, (c + 1) * BS)
        xh = np.empty((H, 2 * BS), dtype=np.float16)
        xh[:, :BS] = x16[sl].T
        xh[:, BS:] = h16[sl].T
        maps.append({
            "xhT": xh,
            "wpack": pack,
            "biasT": biasT,
        })
    return maps


def run(inputs, **kw):
    nc = _program()
    res = run_bass_kernel_spmd(nc, _in_maps(inputs), list(range(NCORES)), **kw)
    out = np.empty((B, H), dtype=np.float32)
    for c in range(NCORES):
        out[c * BS:(c + 1) * BS, :] = res.results[c]["hyT"].T
    return out, res


def kernel(**inputs) -> np.ndarray:
    out, _ = run(inputs)
    return out
